# revision 1
# baseline (speedup 1.0000x reference)
"""BitSSM fused kernel for 8 Trainium2 NeuronCores.

Strategy
--------
Data-parallel over tokens: B*S = 16384 tokens split into 8 shards of 2048.
All ops are token-local except the causal depthwise conv (K=4), whose
3-token left halo is precomputed on the host per shard.

PE does all heavy math (one 512-token moving pass costs the same 216ns
regardless of dtype; fp8 DoubleRow contracts 2 K-planes per pass):
  in_proj : fp16 moving x, fp8 ternary stationary      (8 planes / ct-tile)
  conv    : 4 shifted fp16 matmuls w/ diagonal stationary, accumulated
            into one PSUM group (K=4 causal depthwise conv)
  x_proj  : fp8 DoubleRow over X8 = fp8(64*xc)         (16 planes -> 8 MMs)
  out_proj: fp8 DoubleRow over X8 and B'               (32 planes -> 16 MMs)
            where B' = fp8(128*xc*gate - X8), so X8 + B' = 128*y with only
            a small-residual fp8 quantization error.

Phases per core (token halves H=1024 keep SBUF under budget):
  A(h): in_proj + conv + silu -> xc (fp16, stored) -> X8 (fp8, stored)
  B(h): x_proj -> gate = sigmoid(s_x/64 * psum + bx) -> t = xc*g ->
        B' = fp8(128*t - X8)
  C   : out_proj over (X8, B') pairs; out = Identity(s_out/128*psum + bo)
"""

import sys

if '/opt/trn_rl_repo' not in sys.path:
    sys.path.insert(0, '/opt/trn_rl_repo')

import numpy as np
import ml_dtypes

D_MODEL, D_STATE, D_INNER = 1024, 16, 2048
EPS = 1e-5
B, S = 4, 4096
N_CORES = 8
T = (B * S) // N_CORES          # tokens per core (2048)
H = T // 2                      # tokens per phase half (1024)
W = 512                         # psum tile width (tokens)
KI = D_MODEL // 128             # 8 contraction planes for in_proj
KC = D_INNER // 128             # 16 contraction planes for x/out_proj
CT = D_INNER // 128             # 16 channel planes of d_inner
DT = D_MODEL // 128             # 8 channel planes of d_model
SC = 64.0                       # fp8 scale for xc

_BUILD_CACHE = {}


def _build(s_x: float, s_out: float):
    import concourse.tile as tile
    from concourse import bacc, mybir

    nc = bacc.Bacc("TRN2", target_bir_lowering=False, debug=False)
    f32 = mybir.dt.float32
    fp16 = mybir.dt.float16
    bf16 = mybir.dt.bfloat16
    fp8 = mybir.dt.float8e4
    AF = mybir.ActivationFunctionType
    ALU = mybir.AluOpType
    DR = mybir.MatmulPerfMode.DoubleRow

    x16_d = nc.dram_tensor("x16", [128, KI, T], fp16, kind="ExternalInput")
    wi_d = nc.dram_tensor("wi", [128, KI * D_INNER], fp8, kind="ExternalInput")
    wx_d = nc.dram_tensor("wx", [128, KC * D_INNER], fp8, kind="ExternalInput")
    wo_d = nc.dram_tensor("wo", [128, KC * D_MODEL], fp8, kind="ExternalInput")
    dg_d = nc.dram_tensor("dg", [128, CT * 3 * 128], fp16, kind="ExternalInput")
    wc_d = nc.dram_tensor("wc", [128, CT * 4], f32, kind="ExternalInput")
    bc_d = nc.dram_tensor("bc", [128, CT], f32, kind="ExternalInput")
    bx_d = nc.dram_tensor("bx", [128, CT], f32, kind="ExternalInput")
    bo_d = nc.dram_tensor("bo", [128, DT], f32, kind="ExternalInput")
    h0_d = nc.dram_tensor("h0", [128, CT * 3], f32, kind="ExternalInput")
    out_d = nc.dram_tensor("out", [128, DT * T], bf16, kind="ExternalOutput")

    with tile.TileContext(nc) as tc:
        with (
            tc.tile_pool(name="wx", bufs=1) as wxpool,
            tc.tile_pool(name="x8", bufs=1) as x8pool,
            tc.tile_pool(name="consts", bufs=1) as cpool,
            tc.tile_pool(name="ps", bufs=8, space="PSUM") as pspool,
        ):
            wx_t = wxpool.tile([128, KC, D_INNER], fp8, name="wx_t")
            X8_t = x8pool.tile([128, KC, T], fp8, name="X8_t")
            BP_t = x8pool.tile([128, KC, T], fp8, name="BP_t")

            with (
                tc.tile_pool(name="xin", bufs=1) as xinpool,
                tc.tile_pool(name="wi", bufs=1) as wipool,
                tc.tile_pool(name="dg", bufs=1) as dgpool,
                tc.tile_pool(name="xi", bufs=2) as xipool,
                tc.tile_pool(name="tap", bufs=4) as tappool,
                tc.tile_pool(name="xc", bufs=1) as xcpool,
                tc.tile_pool(name="gate", bufs=2) as gatepool,
                tc.tile_pool(name="tw", bufs=2) as twpool,
            ):
                wi_t = wipool.tile([128, KI, D_INNER], fp8, name="wi_t")
                dg_t = dgpool.tile([128, CT * 3, 128], fp16, name="dg_t")
                wc_t = cpool.tile([128, CT * 4], f32, name="wc_t")
                bc_t = cpool.tile([128, CT], f32, name="bc_t")
                bx_t = cpool.tile([128, CT], f32, name="bx_t")
                bo_t = cpool.tile([128, DT], f32, name="bo_t")
                h0_t = cpool.tile([128, CT * 3], f32, name="h0_t")
                halo1 = cpool.tile([128, CT * 3], fp16, name="halo1")

                def phase_A(h):
                    x16_t = xinpool.tile([128, KI, H], fp16, tag="x16",
                                         name=f"x16_{h}")
                    if h == 0:
                        # plane-interleaved loads so matmuls start early
                        for kt in range(KI):
                            nc.sync.dma_start(
                                wi_t[:, kt, :],
                                wi_d[:, kt * D_INNER:(kt + 1) * D_INNER])
                            nc.sync.dma_start(
                                x16_t[:, kt, :], x16_d[:, kt, 0:H])
                        nc.sync.dma_start(dg_t[:], dg_d[:, :])
                        nc.sync.dma_start(wc_t[:], wc_d[:, :])
                        nc.sync.dma_start(bc_t[:], bc_d[:, :])
                        nc.sync.dma_start(bx_t[:], bx_d[:, :])
                        nc.sync.dma_start(bo_t[:], bo_d[:, :])
                        nc.sync.dma_start(h0_t[:], h0_d[:, :])
                        nc.sync.dma_start(wx_t[:], wx_d[:, :])
                    else:
                        nc.sync.dma_start(
                            x16_t[:], x16_d[:, :, H:2 * H])
                    phase_A.xc_t = xcpool.tile([128, CT, H], fp16, tag="xc",
                                               name=f"xc_{h}")
                    pend = []          # (ct, in_ps pair) awaiting conv

                    def conv_group(ct, ps_in):
                        xi_t = xipool.tile([128, 3 + H], fp16, tag="xi",
                                           name=f"xi{ct}_{h}")
                        if h == 0:
                            nc.vector.tensor_copy(
                                xi_t[:, 0:3], h0_t[:, ct * 3:ct * 3 + 3])
                        else:
                            nc.vector.tensor_copy(
                                xi_t[:, 0:3], halo1[:, ct * 3:ct * 3 + 3])
                        for jh in range(2):
                            nc.vector.tensor_copy(
                                xi_t[:, 3 + jh * W:3 + (jh + 1) * W],
                                ps_in[jh][:])
                        if h == 0:
                            nc.vector.tensor_copy(
                                halo1[:, ct * 3:ct * 3 + 3],
                                xi_t[:, H:H + 3])
                        ps_cv = [pspool.tile([128, W], f32, tag="ps",
                                             name=f"pscv{ct}_{jh}_{h}")
                                 for jh in range(2)]
                        for k in range(3):
                            dsl = dg_t[:, ct * 3 + k, :]
                            for jh in range(2):
                                nc.tensor.matmul(
                                    ps_cv[jh][:], dsl,
                                    xi_t[:, k + jh * W:k + jh * W + W],
                                    start=(k == 0), stop=(k == 2))
                        for jh in range(2):
                            tap_t = tappool.tile([128, W], fp16, tag="tap",
                                                 name=f"tap{ct}_{jh}_{h}")
                            nc.vector.scalar_tensor_tensor(
                                tap_t[:], xi_t[:, 3 + jh * W:3 + jh * W + W],
                                wc_t[:, ct * 4 + 3:ct * 4 + 4], ps_cv[jh][:],
                                op0=ALU.mult, op1=ALU.add)
                            nc.scalar.activation(
                                phase_A.xc_t[:, ct, jh * W:(jh + 1) * W],
                                tap_t[:], AF.Silu,
                                bias=bc_t[:, ct:ct + 1], scale=1.0)
                        nc.vector.tensor_scalar_mul(
                            X8_t[:, ct, h * H:(h + 1) * H],
                            phase_A.xc_t[:, ct, :], SC)

                    for ctb in range(0, CT, 2):
                        ps_blk = {}
                        for ct in (ctb, ctb + 1):
                            ps_blk[ct] = [
                                pspool.tile([128, W], f32, tag="ps",
                                            name=f"psin{ct}_{jh}_{h}")
                                for jh in range(2)]
                        for kt in range(KI):
                            for ct in (ctb, ctb + 1):
                                wsl = wi_t[:, kt, ct * 128:(ct + 1) * 128]
                                for jh in range(2):
                                    nc.tensor.matmul(
                                        ps_blk[ct][jh][:], wsl,
                                        x16_t[:, kt, jh * W:(jh + 1) * W],
                                        start=(kt == 0), stop=(kt == KI - 1))
                        for ct in (ctb, ctb + 1):
                            pend.append((ct, ps_blk[ct]))
                        while len(pend) > 2:
                            conv_group(*pend.pop(0))
                    while pend:
                        conv_group(*pend.pop(0))

                def phase_B(h):
                    xc_t = phase_A.xc_t
                    for c2 in range(CT):
                        ps_j = [pspool.tile([128, W], f32, tag="ps",
                                            name=f"psb{c2}_{jh}_{h}")
                                for jh in range(2)]
                        for kp in range(KC // 2):
                            wsl = wx_t[:, 2 * kp:2 * kp + 2,
                                       c2 * 128:(c2 + 1) * 128]
                            for jh in range(2):
                                nc.tensor.matmul(
                                    ps_j[jh][:], wsl,
                                    X8_t[:, 2 * kp:2 * kp + 2,
                                         h * H + jh * W:h * H + (jh + 1) * W],
                                    start=(kp == 0), stop=(kp == KC // 2 - 1),
                                    perf_mode=DR)
                        gate_t = gatepool.tile([128, H], fp16, tag="g",
                                               name=f"g{c2}_{h}")
                        for jh in range(2):
                            nc.scalar.activation(
                                gate_t[:, jh * W:(jh + 1) * W], ps_j[jh][:],
                                AF.Sigmoid, bias=bx_t[:, c2:c2 + 1],
                                scale=s_x / SC)
                        t_t = twpool.tile([128, H], fp16, tag="t",
                                          name=f"t{c2}_{h}")
                        nc.vector.tensor_tensor(
                            t_t[:], xc_t[:, c2, :], gate_t[:], op=ALU.mult)
                        nc.vector.scalar_tensor_tensor(
                            BP_t[:, c2, h * H:(h + 1) * H], t_t[:], 2 * SC,
                            X8_t[:, c2, h * H:(h + 1) * H],
                            op0=ALU.mult, op1=ALU.subtract)

                phase_A(0)
                phase_B(0)
                phase_A(1)
                phase_B(1)

            # ---- phase C: out_proj ----
            with (
                tc.tile_pool(name="wo", bufs=1) as wopool,
                tc.tile_pool(name="out", bufs=4) as opool,
            ):
                wo_t = wopool.tile([128, KC, D_MODEL], fp8, name="wo_t")
                for kp in range(KC // 2):
                    nc.sync.dma_start(
                        wo_t[:, 2 * kp:2 * kp + 2, :],
                        wo_d[:, 2 * kp * D_MODEL:(2 * kp + 2) * D_MODEL])
                for dt in range(DT):
                    ps_j = [pspool.tile([128, W], f32, tag="ps",
                                        name=f"psc{dt}_{j}") for j in range(4)]
                    for src_i, src in enumerate((X8_t, BP_t)):
                        for kp in range(KC // 2):
                            wsl = wo_t[:, 2 * kp:2 * kp + 2,
                                       dt * 128:(dt + 1) * 128]
                            for j in range(4):
                                nc.tensor.matmul(
                                    ps_j[j][:], wsl,
                                    src[:, 2 * kp:2 * kp + 2,
                                        j * W:(j + 1) * W],
                                    start=(src_i == 0 and kp == 0),
                                    stop=(src_i == 1 and kp == KC // 2 - 1),
                                    perf_mode=DR)
                    for j in range(4):
                        ot = opool.tile([128, W], bf16, tag="ot",
                                        name=f"ot{dt}_{j}")
                        nc.scalar.activation(ot[:], ps_j[j][:], AF.Identity,
                                             bias=bo_t[:, dt:dt + 1],
                                             scale=s_out / (2 * SC))
                        nc.sync.dma_start(
                            out_d[:, dt * T + j * W:dt * T + (j + 1) * W],
                            ot[:])

    nc.compile()
    return nc


def _quantize(w):
    s = np.float32(max(np.abs(w).mean(dtype=np.float64), EPS))
    return np.clip(np.round(w / s), -1.0, 1.0).astype(np.float32), s


def _plane_pack(a, nplanes, width):
    """[nplanes*128, width] -> [128, nplanes*width] with plane-major cols."""
    return np.ascontiguousarray(
        a.reshape(nplanes, 128, width).transpose(1, 0, 2).reshape(
            128, nplanes * width))


def kernel(x, w_in, b_in, w_conv, b_conv, w_x, b_x, w_out, b_out,
           _trace=False, _trace_kwargs=None):
    from concourse import bass_utils

    x = np.asarray(x, dtype=np.float32)
    w_in = np.asarray(w_in, dtype=np.float32)
    b_in = np.asarray(b_in, dtype=np.float32)
    w_conv = np.asarray(w_conv, dtype=np.float32)
    b_conv = np.asarray(b_conv, dtype=np.float32)
    w_x = np.asarray(w_x, dtype=np.float32)
    b_x = np.asarray(b_x, dtype=np.float32)
    w_out = np.asarray(w_out, dtype=np.float32)
    b_out = np.asarray(b_out, dtype=np.float32)

    # ---- host-side BitNet quantization (exact ternary) ----
    wq_in, s_in = _quantize(w_in)
    wq_x, s_x = _quantize(w_x)
    wq_out, s_out = _quantize(w_out)
    wq_in = wq_in[:D_INNER]           # res half unused downstream
    wq_x_d = wq_x[:D_INNER]           # only delta rows used

    fp8 = ml_dtypes.float8_e4m3
    fp16 = np.float16
    wi_pk = _plane_pack(np.ascontiguousarray(wq_in.T), KI, D_INNER).astype(fp8)
    wx_pk = _plane_pack(np.ascontiguousarray(wq_x_d.T), KC,
                        D_INNER).astype(fp8)
    wo_pk = _plane_pack(np.ascontiguousarray(wq_out.T), KC,
                        D_MODEL).astype(fp8)

    # conv taps 0-2 as diagonal [128,128] fp16 stationary blocks; tap 3 runs
    # on DVE as an STT with the per-partition scalar from wc
    wc = (s_in * w_conv[:, 0, :]).astype(np.float32)             # [D_INNER, 4]
    dg = np.zeros((128, CT * 3, 128), dtype=fp16)
    wc16 = wc.astype(fp16)
    for ct in range(CT):
        for k in range(3):
            np.fill_diagonal(dg[:, ct * 3 + k, :], wc16[ct * 128:(ct + 1) * 128, k])
    dg_pk = np.ascontiguousarray(dg.reshape(128, CT * 3 * 128))
    wc_pk = np.ascontiguousarray(
        wc.reshape(CT, 128, 4).transpose(1, 0, 2).reshape(128, CT * 4))

    bc = (b_in[:D_INNER] * w_conv[:, 0, :].sum(axis=1)
          + b_conv).astype(np.float32)
    bc_pk = _plane_pack(bc, CT, 1)
    bx_pk = _plane_pack(b_x[:D_INNER].astype(np.float32), CT, 1)
    bo_pk = _plane_pack(b_out.astype(np.float32), DT, 1)

    # ---- shard inputs: x^T in fp16 ----
    x_flat = x.reshape(B * S, D_MODEL)
    xT = np.ascontiguousarray(x_flat.T)                   # [D_MODEL, B*S] f32
    xT16 = xT.astype(fp16)

    # raw in_proj value that makes x_inner == 0 (sequence-start padding)
    pad_raw = (-b_in[:D_INNER] / s_in).astype(np.float32)

    in_maps = []
    for c in range(N_CORES):
        t0 = c * T
        x16 = _plane_pack(xT16[:, t0:t0 + T], KI, T).reshape(128, KI, T)
        if t0 % S == 0:
            h0 = np.repeat(pad_raw[:, None], 3, axis=1)   # [D_INNER, 3]
        else:
            h0 = wq_in @ x_flat[t0 - 3:t0].T              # [D_INNER, 3]
        h0_pk = _plane_pack(h0.astype(np.float32), CT, 3)
        in_maps.append({
            "x16": x16, "wi": wi_pk, "wx": wx_pk, "wo": wo_pk,
            "dg": dg_pk, "wc": wc_pk, "bc": bc_pk, "bx": bx_pk,
            "bo": bo_pk, "h0": h0_pk,
        })

    key = (float(s_x), float(s_out))
    if key not in _BUILD_CACHE:
        _BUILD_CACHE[key] = _build(float(s_x), float(s_out))
    nc = _BUILD_CACHE[key]

    kwargs = {}
    if _trace:
        kwargs["trace"] = True
        if _trace_kwargs:
            kwargs.update(_trace_kwargs)
    res = bass_utils.run_bass_kernel_spmd(
        nc, in_maps, core_ids=list(range(N_CORES)), **kwargs)
    kernel.last_results = res

    outs = []
    for c in range(N_CORES):
        arr = np.asarray(res.results[c]["out"]).astype(np.float32)
        outs.append(arr.reshape(128, DT, T).transpose(1, 0, 2).reshape(
            D_MODEL, T))
    full = np.concatenate(outs, axis=1)                   # [D_MODEL, B*S]
    return np.ascontiguousarray(full.T).reshape(B, S, D_MODEL).astype(
        np.float32)



# revision 2
# speedup vs baseline: 1.0297x; 1.0297x over previous
"""BitSSM fused kernel for 8 Trainium2 NeuronCores.

Strategy
--------
Data-parallel over tokens: B*S = 16384 tokens split into 8 shards of 2048.
All ops are token-local except the causal depthwise conv (K=4), whose
3-token left halo is precomputed on the host per shard.

PE does only the three GEMMs (each 512-token pass costs ~220ns; fp8
DoubleRow contracts 2 K-planes per pass):
  in_proj : fp16 moving x, fp8 ternary stationary      (8 passes / ct-tile)
  x_proj  : fp8 DoubleRow over X8 = fp8(64*xc)         (16 planes -> 8 MMs)
  out_proj: fp8 DoubleRow over X8 and B'               (32 planes -> 16 MMs)
            where B' = fp8(128*xc*gate - X8), so X8 + B' = 128*y with only
            a small-residual fp8 quantization error.
The K=4 causal depthwise conv runs on DVE as a shifted multiply-accumulate
chain (tensor_scalar + 3x scalar_tensor_tensor with per-partition taps);
PSUM->SBUF staging copies run on the Scalar engine (Identity activation).
A burst of tiny warm-up matmuls during the initial DMA wait brings the PE
out of the HAM 1.2GHz cold state before the real stream starts.

Phases per core (token halves H=1024 keep SBUF under budget):
  A(h): in_proj -> (scalar copy to xi) -> DVE conv -> silu -> xc (fp16)
        -> X8 (fp8, stored)
  B(h): x_proj -> gate = sigmoid(s_x/64 * psum + bx) -> t = xc*g ->
        B' = fp8(128*t - X8)
  C   : out_proj over (X8, B') pairs; out = Identity(s_out/128*psum + bo)
"""

import sys

if '/opt/trn_rl_repo' not in sys.path:
    sys.path.insert(0, '/opt/trn_rl_repo')

import numpy as np
import ml_dtypes

D_MODEL, D_STATE, D_INNER = 1024, 16, 2048
EPS = 1e-5
B, S = 4, 4096
N_CORES = 8
T = (B * S) // N_CORES          # tokens per core (2048)
H = T // 2                      # tokens per phase half (1024)
W = 512                         # psum tile width (tokens)
KI = D_MODEL // 128             # 8 contraction planes for in_proj
KC = D_INNER // 128             # 16 contraction planes for x/out_proj
CT = D_INNER // 128             # 16 channel planes of d_inner
DT = D_MODEL // 128             # 8 channel planes of d_model
SC = 64.0                       # fp8 scale for xc

_BUILD_CACHE = {}


def _build(s_x: float, s_out: float):
    import concourse.tile as tile
    from concourse import bacc, mybir

    nc = bacc.Bacc("TRN2", target_bir_lowering=False, debug=False)
    f32 = mybir.dt.float32
    fp16 = mybir.dt.float16
    fp8 = mybir.dt.float8e4
    AF = mybir.ActivationFunctionType
    ALU = mybir.AluOpType
    DR = mybir.MatmulPerfMode.DoubleRow

    x16_d = nc.dram_tensor("x16", [128, KI, T], fp16, kind="ExternalInput")
    wi_d = nc.dram_tensor("wi", [128, KI * D_INNER], fp8, kind="ExternalInput")
    wx_d = nc.dram_tensor("wx", [128, KC * D_INNER], fp8, kind="ExternalInput")
    wo_d = nc.dram_tensor("wo", [128, KC * D_MODEL], fp8, kind="ExternalInput")
    wc_d = nc.dram_tensor("wc", [128, CT * 4], f32, kind="ExternalInput")
    bc_d = nc.dram_tensor("bc", [128, CT], f32, kind="ExternalInput")
    bx_d = nc.dram_tensor("bx", [128, CT], f32, kind="ExternalInput")
    bo_d = nc.dram_tensor("bo", [128, DT], f32, kind="ExternalInput")
    h0_d = nc.dram_tensor("h0", [128, CT * 3], f32, kind="ExternalInput")
    out_d = nc.dram_tensor("out", [128, DT * T], fp16, kind="ExternalOutput")

    with tile.TileContext(nc) as tc:
        with (
            tc.tile_pool(name="wx", bufs=1) as wxpool,
            tc.tile_pool(name="x8", bufs=1) as x8pool,
            tc.tile_pool(name="consts", bufs=1) as cpool,
            tc.tile_pool(name="ps", bufs=8, space="PSUM") as pspool,
        ):
            wx_t = wxpool.tile([128, KC, D_INNER], fp8, name="wx_t")
            X8_t = x8pool.tile([128, KC, T], fp8, name="X8_t")
            BP_t = x8pool.tile([128, KC, T], fp8, name="BP_t")

            # PE warm-up: small matmuls with no DMA dependency so the HAM
            # clock-gate opens while the first input DMAs are in flight.
            warm = cpool.tile([128, 128], fp16, name="warm")
            nc.vector.memset(warm[:], 0.0)
            ps_w = pspool.tile([128, 128], f32, tag="ps", name="ps_warm")
            for _ in range(24):
                nc.tensor.matmul(ps_w[:], warm[:], warm[:],
                                 start=True, stop=True)

            with (
                tc.tile_pool(name="xin", bufs=1) as xinpool,
                tc.tile_pool(name="wi", bufs=1) as wipool,
                tc.tile_pool(name="xi", bufs=3) as xipool,
                tc.tile_pool(name="acc", bufs=3) as accpool,
                tc.tile_pool(name="tap", bufs=3) as tappool,
                tc.tile_pool(name="xc", bufs=1) as xcpool,
                tc.tile_pool(name="gate", bufs=2) as gatepool,
                tc.tile_pool(name="tw", bufs=2) as twpool,
            ):
                wi_t = wipool.tile([128, KI, D_INNER], fp8, name="wi_t")
                wc_t = cpool.tile([128, CT * 4], f32, name="wc_t")
                bc_t = cpool.tile([128, CT], f32, name="bc_t")
                bx_t = cpool.tile([128, CT], f32, name="bx_t")
                bo_t = cpool.tile([128, DT], f32, name="bo_t")
                h0_t = cpool.tile([128, CT * 3], f32, name="h0_t")
                halo1 = cpool.tile([128, CT * 3], fp16, name="halo1")
                x16_t = [xinpool.tile([128, KI, H], fp16, name=f"x16_{h}")
                         for h in range(2)]

                # critical-path-first DMA order: plane 0 of x and weights
                # unblock the first matmuls; bulk/late tensors follow.
                nc.sync.dma_start(x16_t[0][:, 0, :], x16_d[:, 0, 0:H])
                nc.sync.dma_start(wi_t[:, 0, :], wi_d[:, 0:D_INNER])
                nc.sync.dma_start(wc_t[:], wc_d[:, :])
                nc.sync.dma_start(bc_t[:], bc_d[:, :])
                nc.sync.dma_start(h0_t[:], h0_d[:, :])
                for kt in range(1, KI):
                    nc.sync.dma_start(x16_t[0][:, kt, :], x16_d[:, kt, 0:H])
                    nc.sync.dma_start(
                        wi_t[:, kt, :],
                        wi_d[:, kt * D_INNER:(kt + 1) * D_INNER])
                nc.sync.dma_start(x16_t[1][:], x16_d[:, :, H:2 * H])
                nc.sync.dma_start(bx_t[:], bx_d[:, :])
                nc.sync.dma_start(bo_t[:], bo_d[:, :])
                nc.sync.dma_start(wx_t[:], wx_d[:, :])

                def phase_A(h):
                    xc_t = xcpool.tile([128, CT, H], fp16, tag="xc",
                                       name=f"xc_{h}")
                    phase_A.xc_t = xc_t

                    def conv_group(ct, ps_in):
                        xi_t = xipool.tile([128, 3 + H], fp16, tag="xi",
                                           name=f"xi{ct}_{h}")
                        if h == 0:
                            nc.vector.tensor_copy(
                                xi_t[:, 0:3], h0_t[:, ct * 3:ct * 3 + 3])
                        else:
                            nc.vector.tensor_copy(
                                xi_t[:, 0:3], halo1[:, ct * 3:ct * 3 + 3])
                        for jh in range(2):
                            nc.scalar.activation(
                                xi_t[:, 3 + jh * W:3 + (jh + 1) * W],
                                ps_in[jh][:], AF.Identity, scale=1.0)
                        if h == 0:
                            nc.vector.tensor_copy(
                                halo1[:, ct * 3:ct * 3 + 3],
                                xi_t[:, H:H + 3])
                        # K=4 causal depthwise conv as shift-mult-accumulate
                        acc_t = accpool.tile([128, H], fp16, tag="acc",
                                             name=f"acc{ct}_{h}")
                        nc.vector.tensor_scalar(
                            acc_t[:], xi_t[:, 0:H],
                            wc_t[:, ct * 4:ct * 4 + 1], None, op0=ALU.mult)
                        for k in (1, 2):
                            nc.vector.scalar_tensor_tensor(
                                acc_t[:], xi_t[:, k:k + H],
                                wc_t[:, ct * 4 + k:ct * 4 + k + 1], acc_t[:],
                                op0=ALU.mult, op1=ALU.add)
                        tap_t = tappool.tile([128, H], fp16, tag="tap",
                                             name=f"tap{ct}_{h}")
                        nc.vector.scalar_tensor_tensor(
                            tap_t[:], xi_t[:, 3:3 + H],
                            wc_t[:, ct * 4 + 3:ct * 4 + 4], acc_t[:],
                            op0=ALU.mult, op1=ALU.add)
                        nc.scalar.activation(
                            xc_t[:, ct, :], tap_t[:], AF.Silu,
                            bias=bc_t[:, ct:ct + 1], scale=1.0)
                        nc.vector.tensor_scalar_mul(
                            X8_t[:, ct, h * H:(h + 1) * H], xc_t[:, ct, :], SC)

                    for ct in range(CT):
                        ps_in = [pspool.tile([128, W], f32, tag="ps",
                                             name=f"psin{ct}_{jh}_{h}")
                                 for jh in range(2)]
                        for kt in range(KI):
                            wsl = wi_t[:, kt, ct * 128:(ct + 1) * 128]
                            for jh in range(2):
                                nc.tensor.matmul(
                                    ps_in[jh][:], wsl,
                                    x16_t[h][:, kt, jh * W:(jh + 1) * W],
                                    start=(kt == 0), stop=(kt == KI - 1))
                        conv_group(ct, ps_in)

                def phase_B(h):
                    xc_t = phase_A.xc_t
                    for c2 in range(CT):
                        ps_j = [pspool.tile([128, W], f32, tag="ps",
                                            name=f"psb{c2}_{jh}_{h}")
                                for jh in range(2)]
                        for kp in range(KC // 2):
                            wsl = wx_t[:, 2 * kp:2 * kp + 2,
                                       c2 * 128:(c2 + 1) * 128]
                            for jh in range(2):
                                nc.tensor.matmul(
                                    ps_j[jh][:], wsl,
                                    X8_t[:, 2 * kp:2 * kp + 2,
                                         h * H + jh * W:h * H + (jh + 1) * W],
                                    start=(kp == 0), stop=(kp == KC // 2 - 1),
                                    perf_mode=DR)
                        gate_t = gatepool.tile([128, H], fp16, tag="g",
                                               name=f"g{c2}_{h}")
                        for jh in range(2):
                            nc.scalar.activation(
                                gate_t[:, jh * W:(jh + 1) * W], ps_j[jh][:],
                                AF.Sigmoid, bias=bx_t[:, c2:c2 + 1],
                                scale=s_x / SC)
                        t_t = twpool.tile([128, H], fp16, tag="t",
                                          name=f"t{c2}_{h}")
                        nc.vector.tensor_tensor(
                            t_t[:], xc_t[:, c2, :], gate_t[:], op=ALU.mult)
                        nc.vector.scalar_tensor_tensor(
                            BP_t[:, c2, h * H:(h + 1) * H], t_t[:], 2 * SC,
                            X8_t[:, c2, h * H:(h + 1) * H],
                            op0=ALU.mult, op1=ALU.subtract)

                phase_A(0)
                phase_B(0)
                phase_A(1)
                phase_B(1)

            # ---- phase C: out_proj ----
            with (
                tc.tile_pool(name="wo", bufs=1) as wopool,
                tc.tile_pool(name="out", bufs=4) as opool,
            ):
                wo_t = wopool.tile([128, KC, D_MODEL], fp8, name="wo_t")
                for kp in range(KC // 2):
                    nc.sync.dma_start(
                        wo_t[:, 2 * kp:2 * kp + 2, :],
                        wo_d[:, 2 * kp * D_MODEL:(2 * kp + 2) * D_MODEL])
                for dt in range(DT):
                    for j in range(4):
                        ps_c = pspool.tile([128, W], f32, tag="ps",
                                           name=f"psc{dt}_{j}")
                        for src_i, src in enumerate((X8_t, BP_t)):
                            for kp in range(KC // 2):
                                wsl = wo_t[:, 2 * kp:2 * kp + 2,
                                           dt * 128:(dt + 1) * 128]
                                nc.tensor.matmul(
                                    ps_c[:], wsl,
                                    src[:, 2 * kp:2 * kp + 2,
                                        j * W:(j + 1) * W],
                                    start=(src_i == 0 and kp == 0),
                                    stop=(src_i == 1 and kp == KC // 2 - 1),
                                    perf_mode=DR)
                        ot = opool.tile([128, W], fp16, tag="ot",
                                        name=f"ot{dt}_{j}")
                        nc.scalar.activation(ot[:], ps_c[:], AF.Identity,
                                             bias=bo_t[:, dt:dt + 1],
                                             scale=s_out / (2 * SC))
                        nc.sync.dma_start(
                            out_d[:, dt * T + j * W:dt * T + (j + 1) * W],
                            ot[:])

    nc.compile()
    return nc


def _quantize(w):
    s = np.float32(max(np.abs(w).mean(dtype=np.float64), EPS))
    return np.clip(np.round(w / s), -1.0, 1.0).astype(np.float32), s


def _plane_pack(a, nplanes, width):
    """[nplanes*128, width] -> [128, nplanes*width] with plane-major cols."""
    return np.ascontiguousarray(
        a.reshape(nplanes, 128, width).transpose(1, 0, 2).reshape(
            128, nplanes * width))


def kernel(x, w_in, b_in, w_conv, b_conv, w_x, b_x, w_out, b_out,
           _trace=False, _trace_kwargs=None):
    from concourse import bass_utils

    x = np.asarray(x, dtype=np.float32)
    w_in = np.asarray(w_in, dtype=np.float32)
    b_in = np.asarray(b_in, dtype=np.float32)
    w_conv = np.asarray(w_conv, dtype=np.float32)
    b_conv = np.asarray(b_conv, dtype=np.float32)
    w_x = np.asarray(w_x, dtype=np.float32)
    b_x = np.asarray(b_x, dtype=np.float32)
    w_out = np.asarray(w_out, dtype=np.float32)
    b_out = np.asarray(b_out, dtype=np.float32)

    # ---- host-side BitNet quantization (exact ternary) ----
    wq_in, s_in = _quantize(w_in)
    wq_x, s_x = _quantize(w_x)
    wq_out, s_out = _quantize(w_out)
    wq_in = wq_in[:D_INNER]           # res half unused downstream
    wq_x_d = wq_x[:D_INNER]           # only delta rows used

    fp8 = ml_dtypes.float8_e4m3
    fp16 = np.float16
    wi_pk = _plane_pack(np.ascontiguousarray(wq_in.T), KI, D_INNER).astype(fp8)
    wx_pk = _plane_pack(np.ascontiguousarray(wq_x_d.T), KC,
                        D_INNER).astype(fp8)
    wo_pk = _plane_pack(np.ascontiguousarray(wq_out.T), KC,
                        D_MODEL).astype(fp8)

    # conv taps as per-partition scalars (DVE shift-mult-accumulate)
    wc = (s_in * w_conv[:, 0, :]).astype(np.float32)             # [D_INNER, 4]
    wc_pk = np.ascontiguousarray(
        wc.reshape(CT, 128, 4).transpose(1, 0, 2).reshape(128, CT * 4))

    bc = (b_in[:D_INNER] * w_conv[:, 0, :].sum(axis=1)
          + b_conv).astype(np.float32)
    bc_pk = _plane_pack(bc, CT, 1)
    bx_pk = _plane_pack(b_x[:D_INNER].astype(np.float32), CT, 1)
    bo_pk = _plane_pack(b_out.astype(np.float32), DT, 1)

    # ---- shard inputs: x^T in fp16 ----
    x_flat = x.reshape(B * S, D_MODEL)
    xT = np.ascontiguousarray(x_flat.T)                   # [D_MODEL, B*S] f32
    xT16 = xT.astype(fp16)

    # raw in_proj value that makes x_inner == 0 (sequence-start padding)
    pad_raw = (-b_in[:D_INNER] / s_in).astype(np.float32)

    in_maps = []
    for c in range(N_CORES):
        t0 = c * T
        x16 = _plane_pack(xT16[:, t0:t0 + T], KI, T).reshape(128, KI, T)
        if t0 % S == 0:
            h0 = np.repeat(pad_raw[:, None], 3, axis=1)   # [D_INNER, 3]
        else:
            h0 = wq_in @ x_flat[t0 - 3:t0].T              # [D_INNER, 3]
        h0_pk = _plane_pack(h0.astype(np.float32), CT, 3)
        in_maps.append({
            "x16": x16, "wi": wi_pk, "wx": wx_pk, "wo": wo_pk,
            "wc": wc_pk, "bc": bc_pk, "bx": bx_pk,
            "bo": bo_pk, "h0": h0_pk,
        })

    key = (float(s_x), float(s_out))
    if key not in _BUILD_CACHE:
        _BUILD_CACHE[key] = _build(float(s_x), float(s_out))
    nc = _BUILD_CACHE[key]

    kwargs = {}
    if _trace:
        kwargs["trace"] = True
        if _trace_kwargs:
            kwargs.update(_trace_kwargs)
    res = bass_utils.run_bass_kernel_spmd(
        nc, in_maps, core_ids=list(range(N_CORES)), **kwargs)
    kernel.last_results = res

    outs = []
    for c in range(N_CORES):
        arr = np.asarray(res.results[c]["out"]).astype(np.float32)
        outs.append(arr.reshape(128, DT, T).transpose(1, 0, 2).reshape(
            D_MODEL, T))
    full = np.concatenate(outs, axis=1)                   # [D_MODEL, B*S]
    return np.ascontiguousarray(full.T).reshape(B, S, D_MODEL).astype(
        np.float32)
